# revision 23
# baseline (speedup 1.0000x reference)
"""Bass/Trainium2 kernel for nn_DotProductAttention (B=8, Tq=Tk=2048, D=1024, fp32).

Sharding: one batch element per NeuronCore (8 cores).

Per core:
  S = Q @ K^T          (fp32r PE matmuls; Q^T / K^T built via PE transposes)
  P = softmax(S, -1)   (DVE reduce_max(negate) -> ACT exp(bias=-max, accum_out=sum)
                        -> DVE reciprocal + tensor_scalar mul)
  C = P @ K            (fp32r PE matmuls; P^T via PE transposes, K DMA-fed as fp32r)

Outputs: (context [B,Tq,D], attention_weights [B,Tq,Tk]) - same order as the
reference nn.Module.
"""

import numpy as np
from contextlib import ExitStack

B, TQ, TK, D = 8, 2048, 2048, 1024
QTILES = TQ // 128   # 16 query tiles per core
KC = TK // 128       # 16 key chunks
DC = D // 128        # 8 feature chunks
NS1 = 4              # 512-wide n-slices for matmul1 (S: 2048 wide)
NS2 = 2              # 512-wide n-slices for matmul2 (C: 1024 wide)

_CACHE = {}


def _build():
    from concourse import bacc
    import concourse.tile as tile
    import concourse.mybir as mybir
    from concourse.masks import make_identity

    F32 = mybir.dt.float32
    F32R = mybir.dt.float32r

    nc = bacc.Bacc("TRN2", target_bir_lowering=False, debug=False)
    dec = nc.dram_tensor("decoder_hidden", [TQ, D], F32, kind="ExternalInput").ap()
    enc = nc.dram_tensor("encoder_outputs", [TK, D], F32, kind="ExternalInput").ap()
    attn = nc.dram_tensor("attention_weights", [TQ, TK], F32, kind="ExternalOutput").ap()
    ctx_out = nc.dram_tensor("context", [TQ, D], F32, kind="ExternalOutput").ap()

    with tile.TileContext(nc) as tc, ExitStack() as ctx:
        consts = ctx.enter_context(tc.tile_pool(name="consts", bufs=1))
        big = ctx.enter_context(tc.tile_pool(name="big", bufs=1))
        work = ctx.enter_context(tc.tile_pool(name="work", bufs=2))
        small = ctx.enter_context(tc.tile_pool(name="small", bufs=3))
        ps_s = ctx.enter_context(tc.tile_pool(name="ps_s", bufs=1, space="PSUM"))
        ps_c = ctx.enter_context(tc.tile_pool(name="ps_c", bufs=1, space="PSUM"))
        ps_t = ctx.enter_context(tc.tile_pool(name="ps_t", bufs=2, space="PSUM"))

        ident = consts.tile([128, 128], F32)
        make_identity(nc, ident)
        ident_r = consts.tile([128, 128], F32R)
        nc.scalar.copy(ident_r, ident)

        cp_flip = [0]

        def group_transpose(srcs, dst, rounded=True):
            """Transpose a group of up to 4 [128,128] blocks through one
            [128,512] PSUM tile, then one big PSUM->SBUF copy (alternating
            ACT/DVE) into dst (free size = 128*len(srcs)).

            rounded=True runs the transpose in fp32r mode (1.5 vs 2 cyc/row);
            the destination operands are consumed as fp32r by the matmuls
            either way, so this loses no precision."""
            tdt = F32R if rounded else F32
            tp = ps_t.tile([128, 512], tdt, tag="tp")
            idn = ident_r if rounded else ident
            for j, src in enumerate(srcs):
                nc.tensor.transpose(tp[:, j * 128:(j + 1) * 128], src, idn)
            tp_used = tp[:, : 128 * len(srcs)]
            if cp_flip[0] % 2 == 0:
                nc.scalar.copy(dst, tp_used)
            else:
                nc.vector.tensor_copy(dst, tp_used)
            cp_flip[0] += 1

        # --- K resident in both layouts -------------------------------------
        # kn: natural [k, d] as 16 chunks of [128, 1024] (fp32r, DMA-fed)
        kn = big.tile([128, KC, D], F32R)
        enc_r = enc.bitcast(F32R)
        # all on SP's HWDGE queues (2 chunks per queue): the per-queue FIFO
        # staggers chunk completions so ns-ordered consumption starts early.
        for kc in range(KC):
            nc.sync.dma_start(kn[:, kc, :], enc_r[kc * 128:(kc + 1) * 128, :])
        # kt: transposed [d, k] as 8 chunks of [128, 2048] (fp32r via ACT/DVE copy)
        # built k-chunk-major: each K chunk is transposed as soon as its DMA
        # lands, hiding the 128 transposes under the 8 MiB K load.
        kt = big.tile([128, DC, TK], F32R)

        def build_kt_kchunk(kc):
            for g in range(DC // 4):
                group_transpose(
                    [
                        kn[:, kc, (4 * g + j) * 128:(4 * g + j + 1) * 128]
                        for j in range(4)
                    ],
                    kt[:, 4 * g:4 * g + 4, kc * 128:(kc + 1) * 128],
                )

        # --- main loop over query tiles -------------------------------------
        # Software pipeline: trace order within iteration i is arranged so the
        # PE alternates mm1_i / mm2_{i-1} with transposes filling the softmax
        # latency gap.
        qt_tiles = [None] * QTILES   # transposed Q tiles (fp32r)
        negmaxes = [None] * QTILES   # per-bank -max [128, NS1]
        rec_tiles = [None] * QTILES  # 1/rowsum
        p_tiles = [None] * QTILES    # softmaxed P tiles
        pt_tiles = [None] * QTILES   # transposed P tiles (fp32r)
        cps_tiles = [None] * QTILES  # C psum tiles

        def load_and_transpose_q(i, early=False):
            q_sb = work.tile([128, D], F32R, tag="q_sb")
            dma_eng = nc.gpsimd if early else nc.sync
            dma_eng.dma_start(q_sb, dec.bitcast(F32R)[i * 128:(i + 1) * 128, :])
            qt = work.tile([128, DC, 128], F32R, tag="qt")
            for g in range(DC // 4):
                group_transpose(
                    [q_sb[:, (4 * g + j) * 128:(4 * g + j + 1) * 128] for j in range(4)],
                    qt[:, 4 * g:4 * g + 4, :],
                )
            qt_tiles[i] = qt

        def mm1_bank(s_ps, i, ns):
            qt = qt_tiles[i]
            for dc in range(DC):
                nc.tensor.matmul(
                    s_ps[:, ns * 512:(ns + 1) * 512],
                    qt[:, dc, :],
                    kt[:, dc, ns * 512:(ns + 1) * 512],
                    start=(dc == 0),
                    stop=(dc == DC - 1),
                )

        def bank_negmax(s_ps, i, ns):
            # per-bank -max as soon as the bank's accumulation group stops,
            # overlapping the remaining banks' matmuls
            if negmaxes[i] is None:
                negmaxes[i] = small.tile([128, NS1], F32, tag="negmaxes", name=f"negmaxes_{i}")
            nc.vector.reduce_max(
                negmaxes[i][:, ns:ns + 1], s_ps[:, ns * 512:(ns + 1) * 512],
                axis=mybir.AxisListType.X, negate=True,
            )

        def mm1(i):
            s_ps = ps_s.tile([128, TQ], F32, tag="s_ps")
            for ns in range(NS1):
                mm1_bank(s_ps, i, ns)
                bank_negmax(s_ps, i, ns)
            return s_ps

        def softmax(i, s_ps):
            negmax = small.tile([128, 1], F32, tag="negmax", name=f"negmax_{i}")
            nc.vector.tensor_reduce(
                negmax, negmaxes[i], axis=mybir.AxisListType.X,
                op=mybir.AluOpType.min,
            )
            p_sb = work.tile([128, TK], F32R, tag="p_sb")
            sums = small.tile([128, 1], F32, tag="sums", name=f"sums_{i}")
            nc.scalar.activation(
                p_sb, s_ps, mybir.ActivationFunctionType.Exp,
                bias=negmax, scale=1.0, accum_out=sums,
            )
            rec = small.tile([128, 1], F32, tag="rec", name=f"rec_{i}")
            nc.vector.reciprocal(rec, sums)
            rec_tiles[i] = rec
            p_tiles[i] = p_sb

        def transpose_p(i):
            p_sb = p_tiles[i]
            pts = []
            for g in range(KC // 4):
                ptg = work.tile([128, 4, 128], F32R, tag=f"ptg{g}", name=f"ptg{g}_{i}")
                group_transpose(
                    [p_sb[:, (4 * g + j) * 128:(4 * g + j + 1) * 128] for j in range(4)],
                    ptg,
                )
                pts.append(ptg)
            pt_tiles[i] = pts
            # normalized attention-weights output (off the critical path; the
            # in-place scale orders after the transposes via WAR deps)
            nc.vector.tensor_scalar_mul(p_sb, p_sb, rec_tiles[i])
            nc.sync.dma_start(attn[i * 128:(i + 1) * 128, :], p_sb.bitcast(F32))

        def mm2(i, ns_outer=False):
            c_ps = ps_c.tile([128, D], F32, tag="c_ps")
            pts = pt_tiles[i]
            if ns_outer:
                # bank-at-a-time: lets the tail copy/DMA of bank 0 overlap
                # bank 1's matmuls (used for the last tile)
                for ns in range(NS2):
                    for kc in range(KC):
                        nc.tensor.matmul(
                            c_ps[:, ns * 512:(ns + 1) * 512],
                            pts[kc // 4][:, kc % 4, :],
                            kn[:, kc, ns * 512:(ns + 1) * 512],
                            start=(kc == 0),
                            stop=(kc == KC - 1),
                        )
            else:
                for kc in range(KC):
                    for ns in range(NS2):
                        nc.tensor.matmul(
                            c_ps[:, ns * 512:(ns + 1) * 512],
                            pts[kc // 4][:, kc % 4, :],
                            kn[:, kc, ns * 512:(ns + 1) * 512],
                            start=(kc == 0),
                            stop=(kc == KC - 1),
                        )
            cps_tiles[i] = c_ps

        def store_c(i, split=False):
            c_ps = cps_tiles[i]
            c_sb = work.tile([128, D], F32, tag="c_sb")
            if split:
                for ns in range(NS2):
                    sl = slice(ns * 512, (ns + 1) * 512)
                    nc.scalar.mul(c_sb[:, sl], c_ps[:, sl], rec_tiles[i])
                    nc.sync.dma_start(ctx_out[i * 128:(i + 1) * 128, sl], c_sb[:, sl])
            else:
                nc.scalar.mul(c_sb, c_ps, rec_tiles[i])
                nc.sync.dma_start(ctx_out[i * 128:(i + 1) * 128, :], c_sb)
            cps_tiles[i] = None

        load_and_transpose_q(0, early=True)
        load_and_transpose_q(1, early=True)
        for i in range(QTILES):
            if i == 0:
                # prologue: transpose K chunks as they land; each 512-wide
                # S bank needs only its 4 K chunks, so tile 0's mm1 starts
                # partway through the K load.
                s_ps = ps_s.tile([128, TQ], F32, tag="s_ps")
                for ns in range(NS1):
                    if ns == 0:
                        # finest-grained start: 256-wide half-banks so the
                        # first matmuls begin after only 2 K chunks land
                        for half in range(2):
                            for kc in (4 * ns + 2 * half, 4 * ns + 2 * half + 1):
                                build_kt_kchunk(kc)
                            for dc in range(DC):
                                nc.tensor.matmul(
                                    s_ps[:, half * 256:(half + 1) * 256],
                                    qt_tiles[0][:, dc, :],
                                    kt[:, dc, half * 256:(half + 1) * 256],
                                    start=(dc == 0),
                                    stop=(dc == DC - 1),
                                )
                    else:
                        for kc in range(4 * ns, 4 * ns + 4):
                            build_kt_kchunk(kc)
                        mm1_bank(s_ps, 0, ns)
                    bank_negmax(s_ps, 0, ns)
            else:
                s_ps = mm1(i)
                mm2(i - 1)
            softmax(i, s_ps)
            if i + 2 < QTILES:
                load_and_transpose_q(i + 2, early=True)
            if i > 0:
                store_c(i - 1)
            transpose_p(i)
        mm2(QTILES - 1, ns_outer=True)
        store_c(QTILES - 1, split=True)

    nc.compile()
    return nc


def kernel(decoder_hidden, encoder_outputs):
    from concourse.bass_utils import run_bass_kernel_spmd

    dh = np.ascontiguousarray(np.asarray(decoder_hidden), dtype=np.float32)
    eo = np.ascontiguousarray(np.asarray(encoder_outputs), dtype=np.float32)
    assert dh.shape == (B, TQ, D) and eo.shape == (B, TK, D)

    if "nc" not in _CACHE:
        _CACHE["nc"] = _build()
    nc = _CACHE["nc"]

    in_maps = [
        {"decoder_hidden": dh[b], "encoder_outputs": eo[b]} for b in range(B)
    ]
    res = run_bass_kernel_spmd(nc, in_maps, core_ids=list(range(B)))
    _CACHE["last_results"] = res
    context = np.stack([r["context"] for r in res.results])
    attention_weights = np.stack([r["attention_weights"] for r in res.results])
    return context, attention_weights


# revision 24
# speedup vs baseline: 1.0044x; 1.0044x over previous
"""Bass/Trainium2 kernel for nn_DotProductAttention (B=8, Tq=Tk=2048, D=1024, fp32).

Sharding: one batch element per NeuronCore (8 cores).

Per core:
  S = Q @ K^T          (fp32r PE matmuls; Q^T / K^T built via grouped PE transposes)
  E = exp(S - max)     (per-bank DVE reduce_max(negate) overlapped with the matmuls,
                        one ACT exp pass with bias=-max and accum_out=row sums)
  C = (E @ K) / sum    (fp32r PE matmuls on unnormalized E^T; the 1/sum scale is
                        folded into the C PSUM->SBUF copy; the attention-weights
                        output is normalized off the critical path)

Outputs: (context [B,Tq,D], attention_weights [B,Tq,Tk]) - same order as the
reference nn.Module.
"""

import numpy as np
from contextlib import ExitStack

B, TQ, TK, D = 8, 2048, 2048, 1024
QTILES = TQ // 128   # 16 query tiles per core
KC = TK // 128       # 16 key chunks
DC = D // 128        # 8 feature chunks
NS1 = 4              # 512-wide n-slices for matmul1 (S: 2048 wide)
NS2 = 2              # 512-wide n-slices for matmul2 (C: 1024 wide)

_CACHE = {}


def _build():
    from concourse import bacc
    import concourse.tile as tile
    import concourse.mybir as mybir
    from concourse.masks import make_identity

    F32 = mybir.dt.float32
    F32R = mybir.dt.float32r

    nc = bacc.Bacc("TRN2", target_bir_lowering=False, debug=False)
    dec = nc.dram_tensor("decoder_hidden", [TQ, D], F32, kind="ExternalInput").ap()
    enc = nc.dram_tensor("encoder_outputs", [TK, D], F32, kind="ExternalInput").ap()
    attn = nc.dram_tensor("attention_weights", [TQ, TK], F32, kind="ExternalOutput").ap()
    ctx_out = nc.dram_tensor("context", [TQ, D], F32, kind="ExternalOutput").ap()

    with tile.TileContext(nc) as tc, ExitStack() as ctx:
        consts = ctx.enter_context(tc.tile_pool(name="consts", bufs=1))
        big = ctx.enter_context(tc.tile_pool(name="big", bufs=1))
        work = ctx.enter_context(tc.tile_pool(name="work", bufs=2))
        small = ctx.enter_context(tc.tile_pool(name="small", bufs=3))
        ps_s = ctx.enter_context(tc.tile_pool(name="ps_s", bufs=1, space="PSUM"))
        ps_c = ctx.enter_context(tc.tile_pool(name="ps_c", bufs=1, space="PSUM"))
        ps_t = ctx.enter_context(tc.tile_pool(name="ps_t", bufs=2, space="PSUM"))

        ident = consts.tile([128, 128], F32)
        make_identity(nc, ident)
        ident_r = consts.tile([128, 128], F32R)
        nc.scalar.copy(ident_r, ident)

        cp_flip = [0]

        def group_transpose(srcs, dst, rounded=True):
            """Transpose a group of up to 4 [128,128] blocks through one
            [128,512] PSUM tile, then one big PSUM->SBUF copy (alternating
            ACT/DVE) into dst (free size = 128*len(srcs)).

            rounded=True runs the transpose in fp32r mode (1.5 vs 2 cyc/row);
            the destination operands are consumed as fp32r by the matmuls
            either way, so this loses no precision."""
            tdt = F32R if rounded else F32
            tp = ps_t.tile([128, 512], tdt, tag="tp")
            idn = ident_r if rounded else ident
            for j, src in enumerate(srcs):
                nc.tensor.transpose(tp[:, j * 128:(j + 1) * 128], src, idn)
            tp_used = tp[:, : 128 * len(srcs)]
            if cp_flip[0] % 2 == 0:
                nc.scalar.copy(dst, tp_used)
            else:
                nc.vector.tensor_copy(dst, tp_used)
            cp_flip[0] += 1

        # --- K resident in both layouts -------------------------------------
        # kn: natural [k, d] as 16 chunks of [128, 1024] (fp32r, DMA-fed)
        kn = big.tile([128, KC, D], F32R)
        enc_r = enc.bitcast(F32R)
        # all on SP's HWDGE queues (2 chunks per queue): the per-queue FIFO
        # staggers chunk completions so ns-ordered consumption starts early.
        for kc in range(KC):
            nc.sync.dma_start(kn[:, kc, :], enc_r[kc * 128:(kc + 1) * 128, :])
        # kt: transposed [d, k] as 8 chunks of [128, 2048] (fp32r via ACT/DVE copy)
        # built k-chunk-major: each K chunk is transposed as soon as its DMA
        # lands, hiding the 128 transposes under the 8 MiB K load.
        kt = big.tile([128, DC, TK], F32R)

        def build_kt_kchunk(kc):
            for g in range(DC // 4):
                group_transpose(
                    [
                        kn[:, kc, (4 * g + j) * 128:(4 * g + j + 1) * 128]
                        for j in range(4)
                    ],
                    kt[:, 4 * g:4 * g + 4, kc * 128:(kc + 1) * 128],
                )

        # --- main loop over query tiles -------------------------------------
        # Software pipeline: trace order within iteration i is arranged so the
        # PE alternates mm1_i / mm2_{i-1} with transposes filling the softmax
        # latency gap.
        qt_tiles = [None] * QTILES   # transposed Q tiles (fp32r)
        negmaxes = [None] * QTILES   # per-bank -max [128, NS1]
        rec_tiles = [None] * QTILES  # 1/rowsum
        p_tiles = [None] * QTILES    # softmaxed P tiles
        pt_tiles = [None] * QTILES   # transposed P tiles (fp32r)
        cps_tiles = [None] * QTILES  # C psum tiles

        def load_and_transpose_q(i, early=False):
            q_sb = work.tile([128, D], F32R, tag="q_sb")
            dma_eng = nc.gpsimd if early else nc.sync
            dma_eng.dma_start(q_sb, dec.bitcast(F32R)[i * 128:(i + 1) * 128, :])
            qt = work.tile([128, DC, 128], F32R, tag="qt")
            for g in range(DC // 4):
                group_transpose(
                    [q_sb[:, (4 * g + j) * 128:(4 * g + j + 1) * 128] for j in range(4)],
                    qt[:, 4 * g:4 * g + 4, :],
                )
            qt_tiles[i] = qt

        def mm1_bank(s_ps, i, ns):
            qt = qt_tiles[i]
            for dc in range(DC):
                nc.tensor.matmul(
                    s_ps[:, ns * 512:(ns + 1) * 512],
                    qt[:, dc, :],
                    kt[:, dc, ns * 512:(ns + 1) * 512],
                    start=(dc == 0),
                    stop=(dc == DC - 1),
                )

        def bank_negmax(s_ps, i, ns):
            # per-bank -max as soon as the bank's accumulation group stops,
            # overlapping the remaining banks' matmuls
            if negmaxes[i] is None:
                negmaxes[i] = small.tile([128, NS1], F32, tag="negmaxes", name=f"negmaxes_{i}")
            nc.vector.reduce_max(
                negmaxes[i][:, ns:ns + 1], s_ps[:, ns * 512:(ns + 1) * 512],
                axis=mybir.AxisListType.X, negate=True,
            )

        def mm1(i):
            s_ps = ps_s.tile([128, TQ], F32, tag="s_ps")
            for ns in range(NS1):
                mm1_bank(s_ps, i, ns)
                bank_negmax(s_ps, i, ns)
            return s_ps

        def softmax(i, s_ps):
            negmax = small.tile([128, 1], F32, tag="negmax", name=f"negmax_{i}")
            nc.vector.tensor_reduce(
                negmax, negmaxes[i], axis=mybir.AxisListType.X,
                op=mybir.AluOpType.min,
            )
            p_sb = work.tile([128, TK], F32R, tag="p_sb")
            sums = small.tile([128, 1], F32, tag="sums", name=f"sums_{i}")
            nc.scalar.activation(
                p_sb, s_ps, mybir.ActivationFunctionType.Exp,
                bias=negmax, scale=1.0, accum_out=sums,
            )
            rec = small.tile([128, 1], F32, tag="rec", name=f"rec_{i}")
            nc.vector.reciprocal(rec, sums)
            rec_tiles[i] = rec
            p_tiles[i] = p_sb

        def transpose_p(i):
            p_sb = p_tiles[i]
            pts = []
            for g in range(KC // 4):
                ptg = work.tile([128, 4, 128], F32R, tag=f"ptg{g}", name=f"ptg{g}_{i}")
                group_transpose(
                    [p_sb[:, (4 * g + j) * 128:(4 * g + j + 1) * 128] for j in range(4)],
                    ptg,
                )
                pts.append(ptg)
            pt_tiles[i] = pts
            # normalized attention-weights output (off the critical path; the
            # in-place scale orders after the transposes via WAR deps)
            nc.vector.tensor_scalar_mul(p_sb, p_sb, rec_tiles[i])
            nc.sync.dma_start(attn[i * 128:(i + 1) * 128, :], p_sb.bitcast(F32))

        def mm2(i, ns_outer=False):
            c_ps = ps_c.tile([128, D], F32, tag="c_ps")
            pts = pt_tiles[i]
            if ns_outer:
                # bank-at-a-time: lets the tail copy/DMA of bank 0 overlap
                # bank 1's matmuls (used for the last tile)
                for ns in range(NS2):
                    for kc in range(KC):
                        nc.tensor.matmul(
                            c_ps[:, ns * 512:(ns + 1) * 512],
                            pts[kc // 4][:, kc % 4, :],
                            kn[:, kc, ns * 512:(ns + 1) * 512],
                            start=(kc == 0),
                            stop=(kc == KC - 1),
                        )
            else:
                for kc in range(KC):
                    for ns in range(NS2):
                        nc.tensor.matmul(
                            c_ps[:, ns * 512:(ns + 1) * 512],
                            pts[kc // 4][:, kc % 4, :],
                            kn[:, kc, ns * 512:(ns + 1) * 512],
                            start=(kc == 0),
                            stop=(kc == KC - 1),
                        )
            cps_tiles[i] = c_ps

        def store_c(i, split=False):
            c_ps = cps_tiles[i]
            c_sb = work.tile([128, D], F32, tag="c_sb")
            if split:
                for ns in range(NS2):
                    sl = slice(ns * 512, (ns + 1) * 512)
                    nc.scalar.mul(c_sb[:, sl], c_ps[:, sl], rec_tiles[i])
                    nc.sync.dma_start(ctx_out[i * 128:(i + 1) * 128, sl], c_sb[:, sl])
            else:
                nc.scalar.mul(c_sb, c_ps, rec_tiles[i])
                nc.sync.dma_start(ctx_out[i * 128:(i + 1) * 128, :], c_sb)
            cps_tiles[i] = None

        load_and_transpose_q(0, early=True)
        load_and_transpose_q(1, early=True)
        for i in range(QTILES):
            if i == 0:
                # prologue: transpose K chunks as they land; each 512-wide
                # S bank needs only its 4 K chunks, so tile 0's mm1 starts
                # partway through the K load.
                s_ps = ps_s.tile([128, TQ], F32, tag="s_ps")
                for ns in range(NS1):
                    if ns == 0:
                        # finest-grained start: 256-wide half-banks so the
                        # first matmuls begin after only 2 K chunks land
                        for half in range(2):
                            for kc in (4 * ns + 2 * half, 4 * ns + 2 * half + 1):
                                build_kt_kchunk(kc)
                            for dc in range(DC):
                                nc.tensor.matmul(
                                    s_ps[:, half * 256:(half + 1) * 256],
                                    qt_tiles[0][:, dc, :],
                                    kt[:, dc, half * 256:(half + 1) * 256],
                                    start=(dc == 0),
                                    stop=(dc == DC - 1),
                                )
                    else:
                        for kc in range(4 * ns, 4 * ns + 4):
                            build_kt_kchunk(kc)
                        mm1_bank(s_ps, 0, ns)
                    bank_negmax(s_ps, 0, ns)
            else:
                s_ps = mm1(i)
                mm2(i - 1)
            softmax(i, s_ps)
            if i + 2 < QTILES:
                load_and_transpose_q(i + 2, early=True)
            if i > 0:
                store_c(i - 1)
            transpose_p(i)
        mm2(QTILES - 1, ns_outer=True)
        store_c(QTILES - 1, split=True)

    nc.compile()
    return nc


def kernel(decoder_hidden, encoder_outputs):
    from concourse.bass_utils import run_bass_kernel_spmd

    dh = np.ascontiguousarray(np.asarray(decoder_hidden), dtype=np.float32)
    eo = np.ascontiguousarray(np.asarray(encoder_outputs), dtype=np.float32)
    assert dh.shape == (B, TQ, D) and eo.shape == (B, TK, D)

    if "nc" not in _CACHE:
        _CACHE["nc"] = _build()
    nc = _CACHE["nc"]

    in_maps = [
        {"decoder_hidden": dh[b], "encoder_outputs": eo[b]} for b in range(B)
    ]
    res = run_bass_kernel_spmd(nc, in_maps, core_ids=list(range(B)))
    _CACHE["last_results"] = res
    context = np.stack([r["context"] for r in res.results])
    attention_weights = np.stack([r["attention_weights"] for r in res.results])
    return context, attention_weights


# revision 25
# speedup vs baseline: 1.0095x; 1.0051x over previous
"""Bass/Trainium2 kernel for nn_DotProductAttention (B=8, Tq=Tk=2048, D=1024, fp32).

Sharding: one batch element per NeuronCore (8 cores).

Per core:
  S = Q @ K^T          (fp32r PE matmuls; Q^T / K^T built via grouped PE transposes)
  E = exp(S - max)     (per-bank DVE reduce_max(negate) overlapped with the matmuls,
                        one ACT exp pass with bias=-max and accum_out=row sums)
  C = (E @ K) / sum    (fp32r PE matmuls on unnormalized E^T; the 1/sum scale is
                        folded into the C PSUM->SBUF copy; the attention-weights
                        output is normalized off the critical path)

Outputs: (context [B,Tq,D], attention_weights [B,Tq,Tk]) - same order as the
reference nn.Module.
"""

import numpy as np
from contextlib import ExitStack

B, TQ, TK, D = 8, 2048, 2048, 1024
QTILES = TQ // 128   # 16 query tiles per core
KC = TK // 128       # 16 key chunks
DC = D // 128        # 8 feature chunks
NS1 = 4              # 512-wide n-slices for matmul1 (S: 2048 wide)
NS2 = 2              # 512-wide n-slices for matmul2 (C: 1024 wide)

_CACHE = {}


def _build():
    from concourse import bacc
    import concourse.tile as tile
    import concourse.mybir as mybir
    from concourse.masks import make_identity

    F32 = mybir.dt.float32
    F32R = mybir.dt.float32r

    nc = bacc.Bacc("TRN2", target_bir_lowering=False, debug=False)
    dec = nc.dram_tensor("decoder_hidden", [TQ, D], F32, kind="ExternalInput").ap()
    enc = nc.dram_tensor("encoder_outputs", [TK, D], F32, kind="ExternalInput").ap()
    attn = nc.dram_tensor("attention_weights", [TQ, TK], F32, kind="ExternalOutput").ap()
    ctx_out = nc.dram_tensor("context", [TQ, D], F32, kind="ExternalOutput").ap()

    with tile.TileContext(nc) as tc, ExitStack() as ctx:
        consts = ctx.enter_context(tc.tile_pool(name="consts", bufs=1))
        big = ctx.enter_context(tc.tile_pool(name="big", bufs=1))
        work = ctx.enter_context(tc.tile_pool(name="work", bufs=2))
        small = ctx.enter_context(tc.tile_pool(name="small", bufs=3))
        ps_sa = ctx.enter_context(tc.tile_pool(name="ps_sa", bufs=1, space="PSUM"))
        ps_sb = ctx.enter_context(tc.tile_pool(name="ps_sb", bufs=1, space="PSUM"))
        ps_c = ctx.enter_context(tc.tile_pool(name="ps_c", bufs=1, space="PSUM"))
        ps_t = ctx.enter_context(tc.tile_pool(name="ps_t", bufs=2, space="PSUM"))

        ident = consts.tile([128, 128], F32)
        make_identity(nc, ident)
        ident_r = consts.tile([128, 128], F32R)
        nc.scalar.copy(ident_r, ident)

        cp_flip = [0]

        def group_transpose(srcs, dst, rounded=True):
            """Transpose a group of up to 4 [128,128] blocks through one
            [128,512] PSUM tile, then one big PSUM->SBUF copy (alternating
            ACT/DVE) into dst (free size = 128*len(srcs)).

            rounded=True runs the transpose in fp32r mode (1.5 vs 2 cyc/row);
            the destination operands are consumed as fp32r by the matmuls
            either way, so this loses no precision."""
            tdt = F32R if rounded else F32
            tp = ps_t.tile([128, 512], tdt, tag="tp")
            idn = ident_r if rounded else ident
            for j, src in enumerate(srcs):
                nc.tensor.transpose(tp[:, j * 128:(j + 1) * 128], src, idn)
            tp_used = tp[:, : 128 * len(srcs)]
            if cp_flip[0] % 2 == 0:
                nc.scalar.copy(dst, tp_used)
            else:
                nc.vector.tensor_copy(dst, tp_used)
            cp_flip[0] += 1

        # --- K resident in both layouts -------------------------------------
        # kn: natural [k, d] as 16 chunks of [128, 1024] (fp32r, DMA-fed)
        kn = big.tile([128, KC, D], F32R)
        enc_r = enc.bitcast(F32R)
        # all on SP's HWDGE queues (2 chunks per queue): the per-queue FIFO
        # staggers chunk completions so ns-ordered consumption starts early.
        for kc in range(KC):
            nc.sync.dma_start(kn[:, kc, :], enc_r[kc * 128:(kc + 1) * 128, :])
        # kt: transposed [d, k] as 8 chunks of [128, 2048] (fp32r via ACT/DVE copy)
        # built k-chunk-major: each K chunk is transposed as soon as its DMA
        # lands, hiding the 128 transposes under the 8 MiB K load.
        kt = big.tile([128, DC, TK], F32R)

        def build_kt_kchunk(kc):
            for g in range(DC // 4):
                group_transpose(
                    [
                        kn[:, kc, (4 * g + j) * 128:(4 * g + j + 1) * 128]
                        for j in range(4)
                    ],
                    kt[:, 4 * g:4 * g + 4, kc * 128:(kc + 1) * 128],
                )

        # --- main loop over query tiles -------------------------------------
        # Software pipeline: trace order within iteration i is arranged so the
        # PE alternates mm1_i / mm2_{i-1} with transposes filling the softmax
        # latency gap.
        qt_tiles = [None] * QTILES   # transposed Q tiles (fp32r)
        negmaxes = [None] * QTILES   # per-bank -max [128, NS1]
        rec_tiles = [None] * QTILES  # 1/rowsum
        p_tiles = [None] * QTILES    # softmaxed P tiles
        pt_tiles = [None] * QTILES   # transposed P tiles (fp32r)
        cps_tiles = [None] * QTILES  # C psum tiles

        def load_and_transpose_q(i, early=False):
            q_sb = work.tile([128, D], F32R, tag="q_sb")
            dma_eng = nc.gpsimd if early else nc.sync
            dma_eng.dma_start(q_sb, dec.bitcast(F32R)[i * 128:(i + 1) * 128, :])
            qt = work.tile([128, DC, 128], F32R, tag="qt")
            for g in range(DC // 4):
                group_transpose(
                    [q_sb[:, (4 * g + j) * 128:(4 * g + j + 1) * 128] for j in range(4)],
                    qt[:, 4 * g:4 * g + 4, :],
                )
            qt_tiles[i] = qt

        def mm1_bank(s_half, i, ns):
            # s_half: [128,1024] psum half-tile; bank ns (global 0..3) lands in
            # local slot ns%2 of half ns//2
            qt = qt_tiles[i]
            loc = ns % 2
            for dc in range(DC):
                nc.tensor.matmul(
                    s_half[:, loc * 512:(loc + 1) * 512],
                    qt[:, dc, :],
                    kt[:, dc, ns * 512:(ns + 1) * 512],
                    start=(dc == 0),
                    stop=(dc == DC - 1),
                )

        def bank_negmax(s_half, i, ns):
            # per-bank -max as soon as the bank's accumulation group stops,
            # overlapping the remaining banks' matmuls
            if negmaxes[i] is None:
                negmaxes[i] = small.tile([128, NS1], F32, tag="negmaxes", name=f"negmaxes_{i}")
            loc = ns % 2
            nc.vector.reduce_max(
                negmaxes[i][:, ns:ns + 1], s_half[:, loc * 512:(loc + 1) * 512],
                axis=mybir.AxisListType.X, negate=True,
            )

        def mm1(i):
            s_a = ps_sa.tile([128, TQ // 2], F32, tag="s_a")
            s_b = ps_sb.tile([128, TQ // 2], F32, tag="s_b")
            for ns in range(NS1):
                s_half = s_a if ns < 2 else s_b
                mm1_bank(s_half, i, ns)
                bank_negmax(s_half, i, ns)
            return (s_a, s_b)

        def softmax(i, s_halves):
            negmax = small.tile([128, 1], F32, tag="negmax", name=f"negmax_{i}")
            nc.vector.tensor_reduce(
                negmax, negmaxes[i], axis=mybir.AxisListType.X,
                op=mybir.AluOpType.min,
            )
            p_sb = work.tile([128, TK], F32R, tag="p_sb")
            sums = small.tile([128, 2], F32, tag="sums", name=f"sums_{i}")
            for h, s_half in enumerate(s_halves):
                nc.scalar.activation(
                    p_sb[:, h * (TK // 2):(h + 1) * (TK // 2)],
                    s_half,
                    mybir.ActivationFunctionType.Exp,
                    bias=negmax, scale=1.0, accum_out=sums[:, h:h + 1],
                )
            ssum = small.tile([128, 1], F32, tag="ssum", name=f"ssum_{i}")
            nc.vector.reduce_sum(ssum, sums, axis=mybir.AxisListType.X)
            rec = small.tile([128, 1], F32, tag="rec", name=f"rec_{i}")
            nc.vector.reciprocal(rec, ssum)
            rec_tiles[i] = rec
            p_tiles[i] = p_sb

        def transpose_p(i):
            p_sb = p_tiles[i]
            pts = []
            for g in range(KC // 4):
                ptg = work.tile([128, 4, 128], F32R, tag=f"ptg{g}", name=f"ptg{g}_{i}")
                group_transpose(
                    [p_sb[:, (4 * g + j) * 128:(4 * g + j + 1) * 128] for j in range(4)],
                    ptg,
                )
                pts.append(ptg)
            pt_tiles[i] = pts
            # normalized attention-weights output (off the critical path; the
            # in-place scale orders after the transposes via WAR deps)
            nc.vector.tensor_scalar_mul(p_sb, p_sb, rec_tiles[i])
            nc.sync.dma_start(attn[i * 128:(i + 1) * 128, :], p_sb.bitcast(F32))

        def mm2(i, ns_outer=False):
            c_ps = ps_c.tile([128, D], F32, tag="c_ps")
            pts = pt_tiles[i]
            if ns_outer:
                # bank-at-a-time: lets the tail copy/DMA of bank 0 overlap
                # bank 1's matmuls (used for the last tile)
                for ns in range(NS2):
                    for kc in range(KC):
                        nc.tensor.matmul(
                            c_ps[:, ns * 512:(ns + 1) * 512],
                            pts[kc // 4][:, kc % 4, :],
                            kn[:, kc, ns * 512:(ns + 1) * 512],
                            start=(kc == 0),
                            stop=(kc == KC - 1),
                        )
            else:
                for kc in range(KC):
                    for ns in range(NS2):
                        nc.tensor.matmul(
                            c_ps[:, ns * 512:(ns + 1) * 512],
                            pts[kc // 4][:, kc % 4, :],
                            kn[:, kc, ns * 512:(ns + 1) * 512],
                            start=(kc == 0),
                            stop=(kc == KC - 1),
                        )
            cps_tiles[i] = c_ps

        def store_c(i, split=False):
            c_ps = cps_tiles[i]
            c_sb = work.tile([128, D], F32, tag="c_sb")
            if split:
                for ns in range(NS2):
                    sl = slice(ns * 512, (ns + 1) * 512)
                    nc.scalar.mul(c_sb[:, sl], c_ps[:, sl], rec_tiles[i])
                    nc.sync.dma_start(ctx_out[i * 128:(i + 1) * 128, sl], c_sb[:, sl])
            else:
                nc.scalar.mul(c_sb, c_ps, rec_tiles[i])
                nc.sync.dma_start(ctx_out[i * 128:(i + 1) * 128, :], c_sb)
            cps_tiles[i] = None

        load_and_transpose_q(0, early=True)
        load_and_transpose_q(1, early=True)
        for i in range(QTILES):
            if i == 0:
                # prologue: transpose K chunks as they land; each 512-wide
                # S bank needs only its 4 K chunks, so tile 0's mm1 starts
                # partway through the K load.
                s_a = ps_sa.tile([128, TQ // 2], F32, tag="s_a")
                s_b = ps_sb.tile([128, TQ // 2], F32, tag="s_b")
                for ns in range(NS1):
                    s_half = s_a if ns < 2 else s_b
                    if ns == 0:
                        # finest-grained start: 256-wide quarter-banks so the
                        # first matmuls begin after only 2 K chunks land
                        for half in range(2):
                            for kc in (2 * half, 2 * half + 1):
                                build_kt_kchunk(kc)
                            for dc in range(DC):
                                nc.tensor.matmul(
                                    s_half[:, half * 256:(half + 1) * 256],
                                    qt_tiles[0][:, dc, :],
                                    kt[:, dc, half * 256:(half + 1) * 256],
                                    start=(dc == 0),
                                    stop=(dc == DC - 1),
                                )
                    else:
                        for kc in range(4 * ns, 4 * ns + 4):
                            build_kt_kchunk(kc)
                        mm1_bank(s_half, 0, ns)
                    bank_negmax(s_half, 0, ns)
                s_ps = (s_a, s_b)
            else:
                s_ps = mm1(i)
                mm2(i - 1)
            softmax(i, s_ps)
            if i + 2 < QTILES:
                load_and_transpose_q(i + 2, early=True)
            if i > 0:
                store_c(i - 1)
            transpose_p(i)
        mm2(QTILES - 1, ns_outer=True)
        store_c(QTILES - 1, split=True)

    nc.compile()
    return nc


def kernel(decoder_hidden, encoder_outputs):
    from concourse.bass_utils import run_bass_kernel_spmd

    dh = np.ascontiguousarray(np.asarray(decoder_hidden), dtype=np.float32)
    eo = np.ascontiguousarray(np.asarray(encoder_outputs), dtype=np.float32)
    assert dh.shape == (B, TQ, D) and eo.shape == (B, TK, D)

    if "nc" not in _CACHE:
        _CACHE["nc"] = _build()
    nc = _CACHE["nc"]

    in_maps = [
        {"decoder_hidden": dh[b], "encoder_outputs": eo[b]} for b in range(B)
    ]
    res = run_bass_kernel_spmd(nc, in_maps, core_ids=list(range(B)))
    _CACHE["last_results"] = res
    context = np.stack([r["context"] for r in res.results])
    attention_weights = np.stack([r["attention_weights"] for r in res.results])
    return context, attention_weights


# revision 26
# speedup vs baseline: 1.0244x; 1.0147x over previous
"""Bass/Trainium2 kernel for nn_DotProductAttention (B=8, Tq=Tk=2048, D=1024, fp32).

Sharding: one batch element per NeuronCore (8 cores).

Per core:
  S = Q @ K^T          (fp32r PE matmuls; Q^T / K^T built via grouped PE transposes)
  E = exp(S - max)     (per-bank DVE reduce_max(negate) overlapped with the matmuls,
                        one ACT exp pass with bias=-max and accum_out=row sums)
  C = (E @ K) / sum    (fp32r PE matmuls on unnormalized E^T; the 1/sum scale is
                        folded into the C PSUM->SBUF copy; the attention-weights
                        output is normalized off the critical path)

Outputs: (context [B,Tq,D], attention_weights [B,Tq,Tk]) - same order as the
reference nn.Module.
"""

import numpy as np
from contextlib import ExitStack

B, TQ, TK, D = 8, 2048, 2048, 1024
QTILES = TQ // 128   # 16 query tiles per core
KC = TK // 128       # 16 key chunks
DC = D // 128        # 8 feature chunks
NS1 = 4              # 512-wide n-slices for matmul1 (S: 2048 wide)
NS2 = 2              # 512-wide n-slices for matmul2 (C: 1024 wide)

_CACHE = {}


def _build():
    from concourse import bacc
    import concourse.tile as tile
    import concourse.mybir as mybir
    from concourse.masks import make_identity

    F32 = mybir.dt.float32
    F32R = mybir.dt.float32r

    nc = bacc.Bacc("TRN2", target_bir_lowering=False, debug=False)
    dec = nc.dram_tensor("decoder_hidden", [TQ, D], F32, kind="ExternalInput").ap()
    enc = nc.dram_tensor("encoder_outputs", [TK, D], F32, kind="ExternalInput").ap()
    attn = nc.dram_tensor("attention_weights", [TQ, TK], F32, kind="ExternalOutput").ap()
    ctx_out = nc.dram_tensor("context", [TQ, D], F32, kind="ExternalOutput").ap()

    with tile.TileContext(nc) as tc, ExitStack() as ctx:
        consts = ctx.enter_context(tc.tile_pool(name="consts", bufs=1))
        big = ctx.enter_context(tc.tile_pool(name="big", bufs=1))
        work = ctx.enter_context(tc.tile_pool(name="work", bufs=2))
        small = ctx.enter_context(tc.tile_pool(name="small", bufs=3))
        ps_sa = ctx.enter_context(tc.tile_pool(name="ps_sa", bufs=1, space="PSUM"))
        ps_sb = ctx.enter_context(tc.tile_pool(name="ps_sb", bufs=1, space="PSUM"))
        ps_c = ctx.enter_context(tc.tile_pool(name="ps_c", bufs=1, space="PSUM"))
        ps_t = ctx.enter_context(tc.tile_pool(name="ps_t", bufs=2, space="PSUM"))

        ident = consts.tile([128, 128], F32)
        make_identity(nc, ident)
        ident_r = consts.tile([128, 128], F32R)
        nc.scalar.copy(ident_r, ident)

        cp_flip = [0]

        def group_transpose(srcs, dst, rounded=True):
            """Transpose a group of up to 4 [128,128] blocks through one
            [128,512] PSUM tile, then one big PSUM->SBUF copy (alternating
            ACT/DVE) into dst (free size = 128*len(srcs)).

            rounded=True runs the transpose in fp32r mode (1.5 vs 2 cyc/row);
            the destination operands are consumed as fp32r by the matmuls
            either way, so this loses no precision."""
            tdt = F32R if rounded else F32
            tp = ps_t.tile([128, 512], tdt, tag="tp")
            idn = ident_r if rounded else ident
            for j, src in enumerate(srcs):
                nc.tensor.transpose(tp[:, j * 128:(j + 1) * 128], src, idn)
            tp_used = tp[:, : 128 * len(srcs)]
            if cp_flip[0] % 2 == 0:
                nc.scalar.copy(dst, tp_used)
            else:
                nc.vector.tensor_copy(dst, tp_used)
            cp_flip[0] += 1

        # --- K resident in both layouts -------------------------------------
        # kn: natural [k, d] as 16 chunks of [128, 1024] (fp32r, DMA-fed)
        kn = big.tile([128, KC, D], F32R)
        enc_r = enc.bitcast(F32R)
        # first two Q tiles load ahead of K on the HWDGE queues so the PE has
        # transpose work immediately; K chunks follow 2-per-queue (the FIFO
        # staggers completions so ns-ordered consumption starts early).
        q0_sb = work.tile([128, D], F32R, tag="q_sb")
        nc.sync.dma_start(q0_sb, dec.bitcast(F32R)[0:128, :])
        q1_sb = work.tile([128, D], F32R, tag="q_sb")
        nc.sync.dma_start(q1_sb, dec.bitcast(F32R)[128:256, :])
        for kc in range(KC):
            nc.sync.dma_start(kn[:, kc, :], enc_r[kc * 128:(kc + 1) * 128, :])
        # kt: transposed [d, k] as 8 chunks of [128, 2048] (fp32r via ACT/DVE copy)
        # built k-chunk-major: each K chunk is transposed as soon as its DMA
        # lands, hiding the 128 transposes under the 8 MiB K load.
        kt = big.tile([128, DC, TK], F32R)

        def build_kt_kchunk(kc):
            for g in range(DC // 4):
                group_transpose(
                    [
                        kn[:, kc, (4 * g + j) * 128:(4 * g + j + 1) * 128]
                        for j in range(4)
                    ],
                    kt[:, 4 * g:4 * g + 4, kc * 128:(kc + 1) * 128],
                )

        # --- main loop over query tiles -------------------------------------
        # Software pipeline: trace order within iteration i is arranged so the
        # PE alternates mm1_i / mm2_{i-1} with transposes filling the softmax
        # latency gap.
        qt_tiles = [None] * QTILES   # transposed Q tiles (fp32r)
        negmaxes = [None] * QTILES   # per-bank -max [128, NS1]
        rec_tiles = [None] * QTILES  # 1/rowsum
        p_tiles = [None] * QTILES    # softmaxed P tiles
        pt_tiles = [None] * QTILES   # transposed P tiles (fp32r)
        cps_tiles = [None] * QTILES  # C psum tiles

        def load_and_transpose_q(i, early=False, preloaded=None):
            if preloaded is not None:
                q_sb = preloaded
            else:
                q_sb = work.tile([128, D], F32R, tag="q_sb")
                dma_eng = nc.gpsimd if early else nc.sync
                dma_eng.dma_start(q_sb, dec.bitcast(F32R)[i * 128:(i + 1) * 128, :])
            qt = work.tile([128, DC, 128], F32R, tag="qt")
            for g in range(DC // 4):
                group_transpose(
                    [q_sb[:, (4 * g + j) * 128:(4 * g + j + 1) * 128] for j in range(4)],
                    qt[:, 4 * g:4 * g + 4, :],
                )
            qt_tiles[i] = qt

        def mm1_bank(s_half, i, ns):
            # s_half: [128,1024] psum half-tile; bank ns (global 0..3) lands in
            # local slot ns%2 of half ns//2
            qt = qt_tiles[i]
            loc = ns % 2
            for dc in range(DC):
                nc.tensor.matmul(
                    s_half[:, loc * 512:(loc + 1) * 512],
                    qt[:, dc, :],
                    kt[:, dc, ns * 512:(ns + 1) * 512],
                    start=(dc == 0),
                    stop=(dc == DC - 1),
                )

        def bank_negmax(s_half, i, ns):
            # per-bank -max as soon as the bank's accumulation group stops,
            # overlapping the remaining banks' matmuls
            if negmaxes[i] is None:
                negmaxes[i] = small.tile([128, NS1], F32, tag="negmaxes", name=f"negmaxes_{i}")
            loc = ns % 2
            nc.vector.reduce_max(
                negmaxes[i][:, ns:ns + 1], s_half[:, loc * 512:(loc + 1) * 512],
                axis=mybir.AxisListType.X, negate=True,
            )

        def mm1(i):
            s_a = ps_sa.tile([128, TQ // 2], F32, tag="s_a")
            s_b = ps_sb.tile([128, TQ // 2], F32, tag="s_b")
            for ns in range(NS1):
                s_half = s_a if ns < 2 else s_b
                mm1_bank(s_half, i, ns)
                bank_negmax(s_half, i, ns)
            return (s_a, s_b)

        def softmax(i, s_halves):
            negmax = small.tile([128, 1], F32, tag="negmax", name=f"negmax_{i}")
            nc.vector.tensor_reduce(
                negmax, negmaxes[i], axis=mybir.AxisListType.X,
                op=mybir.AluOpType.min,
            )
            p_sb = work.tile([128, TK], F32R, tag="p_sb")
            sums = small.tile([128, 2], F32, tag="sums", name=f"sums_{i}")
            for h, s_half in enumerate(s_halves):
                nc.scalar.activation(
                    p_sb[:, h * (TK // 2):(h + 1) * (TK // 2)],
                    s_half,
                    mybir.ActivationFunctionType.Exp,
                    bias=negmax, scale=1.0, accum_out=sums[:, h:h + 1],
                )
            ssum = small.tile([128, 1], F32, tag="ssum", name=f"ssum_{i}")
            nc.vector.reduce_sum(ssum, sums, axis=mybir.AxisListType.X)
            rec = small.tile([128, 1], F32, tag="rec", name=f"rec_{i}")
            nc.vector.reciprocal(rec, ssum)
            rec_tiles[i] = rec
            p_tiles[i] = p_sb

        def transpose_p(i):
            p_sb = p_tiles[i]
            pts = []
            for g in range(KC // 4):
                ptg = work.tile([128, 4, 128], F32R, tag=f"ptg{g}", name=f"ptg{g}_{i}")
                group_transpose(
                    [p_sb[:, (4 * g + j) * 128:(4 * g + j + 1) * 128] for j in range(4)],
                    ptg,
                )
                pts.append(ptg)
            pt_tiles[i] = pts
            # normalized attention-weights output (off the critical path; the
            # in-place scale orders after the transposes via WAR deps)
            nc.vector.tensor_scalar_mul(p_sb, p_sb, rec_tiles[i])
            nc.sync.dma_start(attn[i * 128:(i + 1) * 128, :], p_sb.bitcast(F32))

        def mm2(i, ns_outer=False):
            c_ps = ps_c.tile([128, D], F32, tag="c_ps")
            pts = pt_tiles[i]
            if ns_outer:
                # bank-at-a-time: lets the tail copy/DMA of bank 0 overlap
                # bank 1's matmuls (used for the last tile)
                for ns in range(NS2):
                    for kc in range(KC):
                        nc.tensor.matmul(
                            c_ps[:, ns * 512:(ns + 1) * 512],
                            pts[kc // 4][:, kc % 4, :],
                            kn[:, kc, ns * 512:(ns + 1) * 512],
                            start=(kc == 0),
                            stop=(kc == KC - 1),
                        )
            else:
                for kc in range(KC):
                    for ns in range(NS2):
                        nc.tensor.matmul(
                            c_ps[:, ns * 512:(ns + 1) * 512],
                            pts[kc // 4][:, kc % 4, :],
                            kn[:, kc, ns * 512:(ns + 1) * 512],
                            start=(kc == 0),
                            stop=(kc == KC - 1),
                        )
            cps_tiles[i] = c_ps

        def store_c(i, split=False):
            c_ps = cps_tiles[i]
            c_sb = work.tile([128, D], F32, tag="c_sb")
            if split:
                for ns in range(NS2):
                    sl = slice(ns * 512, (ns + 1) * 512)
                    nc.scalar.mul(c_sb[:, sl], c_ps[:, sl], rec_tiles[i])
                    nc.sync.dma_start(ctx_out[i * 128:(i + 1) * 128, sl], c_sb[:, sl])
            else:
                nc.scalar.mul(c_sb, c_ps, rec_tiles[i])
                nc.sync.dma_start(ctx_out[i * 128:(i + 1) * 128, :], c_sb)
            cps_tiles[i] = None

        load_and_transpose_q(0, preloaded=q0_sb)
        load_and_transpose_q(1, preloaded=q1_sb)
        for i in range(QTILES):
            if i == 0:
                # prologue: transpose K chunks as they land; each 512-wide
                # S bank needs only its 4 K chunks, so tile 0's mm1 starts
                # partway through the K load.
                s_a = ps_sa.tile([128, TQ // 2], F32, tag="s_a")
                s_b = ps_sb.tile([128, TQ // 2], F32, tag="s_b")
                for ns in range(NS1):
                    s_half = s_a if ns < 2 else s_b
                    if ns == 0:
                        # finest-grained start: 256-wide quarter-banks so the
                        # first matmuls begin after only 2 K chunks land
                        for half in range(2):
                            for kc in (2 * half, 2 * half + 1):
                                build_kt_kchunk(kc)
                            for dc in range(DC):
                                nc.tensor.matmul(
                                    s_half[:, half * 256:(half + 1) * 256],
                                    qt_tiles[0][:, dc, :],
                                    kt[:, dc, half * 256:(half + 1) * 256],
                                    start=(dc == 0),
                                    stop=(dc == DC - 1),
                                )
                    else:
                        for kc in range(4 * ns, 4 * ns + 4):
                            build_kt_kchunk(kc)
                        mm1_bank(s_half, 0, ns)
                    bank_negmax(s_half, 0, ns)
                s_ps = (s_a, s_b)
            else:
                s_ps = mm1(i)
                mm2(i - 1)
            softmax(i, s_ps)
            if i + 2 < QTILES:
                load_and_transpose_q(i + 2, early=True)
            if i > 0:
                store_c(i - 1)
            transpose_p(i)
        mm2(QTILES - 1, ns_outer=True)
        store_c(QTILES - 1, split=True)

    nc.compile()
    return nc


def kernel(decoder_hidden, encoder_outputs):
    from concourse.bass_utils import run_bass_kernel_spmd

    dh = np.ascontiguousarray(np.asarray(decoder_hidden), dtype=np.float32)
    eo = np.ascontiguousarray(np.asarray(encoder_outputs), dtype=np.float32)
    assert dh.shape == (B, TQ, D) and eo.shape == (B, TK, D)

    if "nc" not in _CACHE:
        _CACHE["nc"] = _build()
    nc = _CACHE["nc"]

    in_maps = [
        {"decoder_hidden": dh[b], "encoder_outputs": eo[b]} for b in range(B)
    ]
    res = run_bass_kernel_spmd(nc, in_maps, core_ids=list(range(B)))
    _CACHE["last_results"] = res
    context = np.stack([r["context"] for r in res.results])
    attention_weights = np.stack([r["attention_weights"] for r in res.results])
    return context, attention_weights
